# revision 3
# baseline (speedup 1.0000x reference)
"""Trainium2 Bass kernel v5 for entmax-1.5 over rows of a masked [8192, 4096] matrix.

Candidate-set Newton (32 candidates/row via fold + chunked MAX8 top-8):
  - fold max(z[:,:2048], z[:,2048:]) runs on GpSimd (Pool) — frees DVE
  - DVE: 4x MAX8@512 per tile -> C[128,32], merge MAX8 -> sorted T8
  - solve: group0 warm-start (closed-form entmax on top-8) + K=2 Newton;
    group1 sqrt-free start + K=3 Newton (all on DVE, high priority)
  - finals: U = relu(z - tau) on DVE tensor_scalar (fp16 4x mode),
    p = U^2 in-place on ACT (Square), single full-tile DMA store
Numpy-validated rel err ~7.9e-3 vs 2e-2 gate.

Sharding: 1024 rows x 8 cores; 8 tiles of [128, 4096] per core.
Self-contained: hardcodes scores[8192,4096] f32 + mask[8192,4096] bool.
"""

import sys

import numpy as np

sys.path.insert(0, "/opt/trn_rl_repo")

N_ROWS = 8192
N_COLS = 4096
N_CORES = 8
P = 128
ROWS_PER_CORE = N_ROWS // N_CORES          # 1024
NT = ROWS_PER_CORE // P                    # 8 tiles per core
NCH = 4
CAND = NCH * 8                             # 32 candidates per row
GROUPS = [(0, 1, 2, 3), (4, 5, 6, 7)]

_CACHE = {}


def build_nc():
    import concourse.bacc as bacc
    import concourse.mybir as mybir
    from concourse.tile import TileContext

    f32 = mybir.dt.float32
    f16 = mybir.dt.float16
    Alu = mybir.AluOpType
    Act = mybir.ActivationFunctionType

    nc = bacc.Bacc("TRN2", target_bir_lowering=False, debug=False)

    z_h = nc.declare_dram_parameter("z", [ROWS_PER_CORE, N_COLS], f16, isOutput=False)
    invk_h = nc.declare_dram_parameter("invk", [P, 8], f32, isOutput=False)
    kvec_h = nc.declare_dram_parameter("kvec", [P, 8], f32, isOutput=False)
    p_h = nc.declare_dram_parameter("p", [ROWS_PER_CORE, N_COLS], f16, isOutput=True)

    z = z_h.ap()
    pout = p_h.ap()
    half = N_COLS // 2
    csz = half // NCH                      # 512

    with TileContext(nc) as tc:
        with (
            tc.tile_pool(name="pt", bufs=NT) as pt,
            tc.tile_pool(name="pw", bufs=3) as pw,
            tc.tile_pool(name="pu", bufs=5) as pu,
            tc.tile_pool(name="ps", bufs=1) as ps,
            tc.tile_pool(name="pq", bufs=4) as pq,
        ):
            invk = ps.tile([P, 8], f32)
            kvec = ps.tile([P, 8], f32)
            tau = ps.tile([P, NT], f32, name="tau")

            t_tiles = [None] * NT
            u_tiles = [None] * NT
            c_tiles = {}
            t8_tiles = {}

            # ---- all input loads up front, in tile order ----
            for i in range(NT):
                t_i = pt.tile([P, N_COLS], f16, name=f"t{i}", tag="t")
                nc.sync.dma_start(out=t_i, in_=z[i * P:(i + 1) * P, :])
                t_tiles[i] = t_i
            nc.sync.dma_start(out=invk, in_=invk_h.ap())
            nc.sync.dma_start(out=kvec, in_=kvec_h.ap())

            def scan_tile(gi, j, i):
                """fold (Pool) + chunked MAX8 (DVE) for tile i, slot j of group gi."""
                C = c_tiles[gi]
                T8 = t8_tiles[gi]
                t_i = t_tiles[i]
                w = pw.tile([P, half], f16, name=f"w{i}", tag="w")
                nc.vector.tensor_tensor(w, t_i[:, :half], t_i[:, half:], Alu.max)
                for c in range(NCH):
                    nc.vector.max(
                        C[:, j * CAND + c * 8: j * CAND + (c + 1) * 8],
                        w[:, c * csz:(c + 1) * csz])
                nc.vector.max(T8[:, j * 8:(j + 1) * 8], C[:, j * CAND:(j + 1) * CAND])

            def phase_scan(gi):
                tiles = GROUPS[gi]
                g = len(tiles)
                C = ps.tile([P, g * CAND], f16, name=f"C{gi}")
                T8 = ps.tile([P, g * 8], f32, name=f"T8_{gi}")
                c_tiles[gi] = C
                t8_tiles[gi] = T8
                for j, i in enumerate(tiles):
                    scan_tile(gi, j, i)

            def phase_solve(gi, k_newton, warm):
                tiles = GROUPS[gi]
                g = len(tiles)
                j0 = tiles[0]
                C = c_tiles[gi]
                T8 = t8_tiles[gi]
                tslice = tau[:, j0:j0 + g]
                sh3 = [P, g, 8]
                T3 = T8.rearrange("p (g k) -> p g k", g=g)
                invk_b = invk.rearrange("p (o k) -> p o k", o=1).broadcast_to(sh3)
                kvec_b = kvec.rearrange("p (o k) -> p o k", o=1).broadcast_to(sh3)
                hp = tc.high_priority()
                hp.__enter__()
                if not warm:
                    # tau0 = max(c1 - 1, (c1+c2)/2 - sqrt(1/2)) — sqrt-free lower bound
                    tmp = pq.tile([P, g], f32, name=f"t0a_{gi}", tag=f"t0a_{gi}")
                    nc.vector.tensor_tensor(
                        tmp.rearrange("p (g o) -> p g o", o=1),
                        T3[:, :, 0:1], T3[:, :, 1:2], Alu.add)
                    nc.vector.tensor_scalar(tmp, tmp, 0.5, -0.70710678,
                                            Alu.mult, Alu.add)
                    nc.vector.tensor_scalar(
                        tslice, T3[:, :, 0], -1.0, None, Alu.add)
                    nc.vector.tensor_tensor(tslice, tslice, tmp, Alu.max)
                else:
                    # closed-form entmax on sorted top-8 warm start
                    q8 = pq.tile(sh3, f32, name=f"q8_{gi}", tag=f"q8_{gi}")
                    nc.vector.tensor_tensor(q8, T3, T3, Alu.mult)

                    def cumsum8(src_, pref):
                        a1 = pq.tile(sh3, f32, name=f"{pref}a_{gi}", tag=f"{pref}a_{gi}")
                        nc.vector.tensor_copy(a1[:, :, 0:1], src_[:, :, 0:1])
                        nc.vector.tensor_tensor(a1[:, :, 1:8], src_[:, :, 1:8], src_[:, :, 0:7], Alu.add)
                        a2 = pq.tile(sh3, f32, name=f"{pref}b_{gi}", tag=f"{pref}b_{gi}")
                        nc.vector.tensor_copy(a2[:, :, 0:2], a1[:, :, 0:2])
                        nc.vector.tensor_tensor(a2[:, :, 2:8], a1[:, :, 2:8], a1[:, :, 0:6], Alu.add)
                        a4 = pq.tile(sh3, f32, name=f"{pref}c_{gi}", tag=f"{pref}c_{gi}")
                        nc.vector.tensor_copy(a4[:, :, 0:4], a2[:, :, 0:4])
                        nc.vector.tensor_tensor(a4[:, :, 4:8], a2[:, :, 4:8], a2[:, :, 0:4], Alu.add)
                        return a4

                    cs = cumsum8(T3, "cs")
                    cq = cumsum8(q8, "cq")
                    mean = pq.tile(sh3, f32, name=f"mean_{gi}", tag=f"mean_{gi}")
                    nc.vector.tensor_tensor(mean, cs, invk_b, Alu.mult)
                    mm = pq.tile(sh3, f32, name=f"mm_{gi}", tag=f"mm_{gi}")
                    nc.vector.tensor_tensor(mm, cq, invk_b, Alu.mult)
                    m2 = pq.tile(sh3, f32, name=f"m2_{gi}", tag=f"m2_{gi}")
                    nc.vector.tensor_tensor(m2, mean, mean, Alu.mult)
                    nc.vector.tensor_tensor(m2, mm, m2, Alu.subtract)
                    nc.vector.tensor_tensor(m2, m2, kvec_b, Alu.mult)
                    nc.vector.tensor_scalar(m2, m2, -1.0, 1.0, Alu.mult, Alu.add)
                    nc.vector.tensor_tensor(m2, m2, invk_b, Alu.mult)
                    nc.vector.tensor_scalar(m2, m2, 0.0, None, Alu.max)
                    sq = pq.tile(sh3, f32, name=f"sq_{gi}", tag=f"sq_{gi}")
                    nc.scalar.sqrt(sq, m2)
                    tauc = pq.tile(sh3, f32, name=f"tauc_{gi}", tag=f"tauc_{gi}")
                    nc.vector.tensor_tensor(tauc, mean, sq, Alu.subtract)
                    ind = pq.tile(sh3, f32, name=f"ind_{gi}", tag=f"ind_{gi}")
                    nc.vector.tensor_tensor(ind, tauc, T3, Alu.is_le)
                    sel = pq.tile(sh3, f32, name=f"sel_{gi}", tag=f"sel_{gi}")
                    nc.vector.tensor_copy(sel[:, :, 7:8], ind[:, :, 7:8])
                    nc.vector.tensor_tensor(sel[:, :, 0:7], ind[:, :, 0:7], ind[:, :, 1:8], Alu.subtract)
                    nc.vector.tensor_tensor(tauc, tauc, sel, Alu.mult)
                    nc.vector.reduce_sum(
                        tslice.rearrange("p (g o) -> p g o", o=1),
                        tauc, axis=mybir.AxisListType.X)
                for it in range(k_newton):
                    U = pq.tile([P, g * CAND], f16, name=f"U{gi}_{it}", tag="U")
                    for j, i in enumerate(tiles):
                        nc.vector.tensor_scalar(
                            U[:, j * CAND:(j + 1) * CAND],
                            C[:, j * CAND:(j + 1) * CAND], tau[:, i:i + 1], 0.0,
                            Alu.subtract, Alu.max)
                    SQ = pq.tile([P, g * CAND], f16, name=f"SQ{gi}_{it}", tag="SQ")
                    nc.vector.tensor_tensor(SQ, U, U, Alu.mult)
                    hF = pq.tile([P, 2 * g], f32, name=f"hF{gi}_{it}", tag="hF")
                    nc.vector.reduce_sum(
                        hF[:, 0:g].rearrange("p (g o) -> p g o", o=1),
                        U.rearrange("p (g c) -> p g c", g=g),
                        axis=mybir.AxisListType.X)
                    nc.vector.reduce_sum(
                        hF[:, g:2 * g].rearrange("p (g o) -> p g o", o=1),
                        SQ.rearrange("p (g c) -> p g c", g=g),
                        axis=mybir.AxisListType.X)
                    num = pq.tile([P, g], f32, name=f"num{gi}_{it}", tag="num")
                    nc.vector.tensor_scalar(num, hF[:, g:2 * g], -1.0, 0.5,
                                            Alu.add, Alu.mult)
                    rd = pq.tile([P, g], f32, name=f"rd{gi}_{it}", tag="rd")
                    nc.vector.reciprocal(rd, hF[:, 0:g])
                    nc.vector.tensor_tensor(num, num, rd, Alu.mult)
                    nc.vector.tensor_tensor(tslice, tslice, num, Alu.add)
                hp.__exit__(None, None, None)

            def final_tile(i):
                """U = relu(z - tau) on DVE (fp16 4x), square in-place on ACT, store."""
                t_i = t_tiles[i]
                u = pu.tile([P, N_COLS], f16, name=f"u{i}", tag="u")
                u_tiles[i] = u
                nc.vector.tensor_scalar(u, t_i, tau[:, i:i + 1], 0.0,
                                        Alu.subtract, Alu.max)
                nc.scalar.activation(u, u, Act.Square)
                nc.sync.dma_start(out=pout[i * P:(i + 1) * P, :], in_=u)

            phase_scan(0)
            s0 = phase_solve(0, k_newton=2, warm=True)
            final_tile(0)
            final_tile(1)
            phase_scan(1)
            final_tile(2)
            final_tile(3)
            s1 = phase_solve(1, k_newton=3, warm=False)
            for i in GROUPS[1]:
                final_tile(i)

    nc.compile()
    return nc


def _host_prep(scores, mask):
    s = np.asarray(scores, dtype=np.float32)
    zq = (np.float32(0.5) * s).astype(np.float16)
    z16 = np.where(np.asarray(mask), zq, np.float16(-4.0))
    k = np.arange(1, 9, dtype=np.float32)
    invk = np.tile(np.float32(1.0) / k, (P, 1)).astype(np.float32)
    kvec = np.tile(k, (P, 1)).astype(np.float32)
    return z16, invk, kvec


def run(scores: np.ndarray, mask: np.ndarray, trace: bool = False, **kw):
    from concourse.bass_utils import run_bass_kernel_spmd

    assert scores.shape == (N_ROWS, N_COLS) and mask.shape == (N_ROWS, N_COLS)
    if "nc" not in _CACHE:
        _CACHE["nc"] = build_nc()
    nc = _CACHE["nc"]

    z16, invk, kvec = _host_prep(scores, mask)
    rpc = ROWS_PER_CORE
    in_maps = [
        {"z": np.ascontiguousarray(z16[i * rpc:(i + 1) * rpc]),
         "invk": invk, "kvec": kvec}
        for i in range(N_CORES)
    ]
    res = run_bass_kernel_spmd(nc, in_maps, list(range(N_CORES)), trace=trace, **kw)
    out = np.concatenate([res.results[i]["p"] for i in range(N_CORES)], axis=0)
    return np.ascontiguousarray(out.astype(np.float32)), res


def kernel(scores: np.ndarray, mask: np.ndarray) -> np.ndarray:
    return run(scores, mask)[0]


if __name__ == "__main__":
    rng = np.random.default_rng(0)
    scores = rng.standard_normal((N_ROWS, N_COLS), dtype=np.float32)
    mask = rng.integers(0, 2, (N_ROWS, N_COLS)).astype(bool)
    out = kernel(scores, mask)
    print("out", out.shape, out.dtype, "rowsum", out.sum(-1)[:4])


# revision 4
# speedup vs baseline: 1.0233x; 1.0233x over previous
"""Trainium2 Bass kernel v6 for entmax-1.5 over rows of a masked [8192, 4096] matrix.

Candidate-set Newton (32 candidates/row via fold + chunked MAX8 top-8):
  - DVE: fold max(zL,zR) (fp16 2x), 4x MAX8@512 per tile -> C[128,32],
    merge MAX8 -> sorted T8 (c1,c2 for the start)
  - solve per group of 4 tiles: tau0 = max(c1-1, (c1+c2)/2 - sqrt(1/2))
    + K=3 Newton over candidates (pure DVE, high priority, no ACT deps)
  - finals: U = relu(z - tau) on DVE tensor_scalar (fp16 4x mode),
    p = U^2 in-place on ACT (Square), full-tile DMA store
  - ALL loads/stores issued from the Pool queue (SWDGE: ~0.34ns/descriptor
    vs ~25ns on HWDGE) so DMA streams at full HBM rate in tile order
Numpy-validated rel err ~7.9e-3 vs 2e-2 gate.

Sharding: 1024 rows x 8 cores; 8 tiles of [128, 4096] per core.
Self-contained: hardcodes scores[8192,4096] f32 + mask[8192,4096] bool.
"""

import sys

import numpy as np

sys.path.insert(0, "/opt/trn_rl_repo")

N_ROWS = 8192
N_COLS = 4096
N_CORES = 8
P = 128
ROWS_PER_CORE = N_ROWS // N_CORES          # 1024
NT = ROWS_PER_CORE // P                    # 8 tiles per core
NCH = 4
CAND = NCH * 8                             # 32 candidates per row
GROUPS = [(0, 1, 2, 3), (4, 5, 6, 7)]

_CACHE = {}


def build_nc():
    import concourse.bacc as bacc
    import concourse.mybir as mybir
    from concourse.tile import TileContext
    from concourse.tile_rust import add_dep_helper

    def _raw(x):
        for attr in ("ins", "instruction", "inst"):
            if hasattr(x, attr):
                return getattr(x, attr)
        return x

    f32 = mybir.dt.float32
    f16 = mybir.dt.float16
    Alu = mybir.AluOpType
    Act = mybir.ActivationFunctionType

    nc = bacc.Bacc("TRN2", target_bir_lowering=False, debug=False)

    z_h = nc.declare_dram_parameter("z", [ROWS_PER_CORE, N_COLS], f16, isOutput=False)
    p_h = nc.declare_dram_parameter("p", [ROWS_PER_CORE, N_COLS], f16, isOutput=True)

    z = z_h.ap()
    pout = p_h.ap()
    half = N_COLS // 2
    csz = half // NCH                      # 512

    with TileContext(nc) as tc:
        with (
            tc.tile_pool(name="pt", bufs=NT) as pt,
            tc.tile_pool(name="pw", bufs=3) as pw,
            tc.tile_pool(name="pu", bufs=5) as pu,
            tc.tile_pool(name="ps", bufs=1) as ps,
            tc.tile_pool(name="pq", bufs=4) as pq,
        ):
            tau = ps.tile([P, NT], f32, name="tau")

            t_tiles = [None] * NT
            c_tiles = {}
            t8_tiles = {}

            # ---- all input loads up front, in tile order, on the Pool queue ----
            for i in range(NT):
                t_i = pt.tile([P, N_COLS], f16, name=f"t{i}", tag="t")
                nc.gpsimd.dma_start(out=t_i, in_=z[i * P:(i + 1) * P, :])
                t_tiles[i] = t_i

            def scan_tile(gi, j, i, after=None):
                """fold + chunked MAX8 (all DVE) for tile i, slot j of group gi."""
                C = c_tiles[gi]
                T8 = t8_tiles[gi]
                t_i = t_tiles[i]
                w = pw.tile([P, half], f16, name=f"w{i}", tag="w")
                fold = nc.vector.tensor_tensor(w, t_i[:, :half], t_i[:, half:], Alu.max)
                if after is not None:
                    add_dep_helper(_raw(fold), _raw(after), sync=False,
                                   reason="pipeline order hint")
                for c in range(NCH):
                    nc.vector.max(
                        C[:, j * CAND + c * 8: j * CAND + (c + 1) * 8],
                        w[:, c * csz:(c + 1) * csz])
                nc.vector.max(T8[:, j * 8:(j + 1) * 8], C[:, j * CAND:(j + 1) * CAND])

            def group_alloc(gi):
                g = len(GROUPS[gi])
                c_tiles[gi] = ps.tile([P, g * CAND], f16, name=f"C{gi}")
                t8_tiles[gi] = ps.tile([P, g * 8], f32, name=f"T8_{gi}")

            def phase_solve(gi, k_newton):
                tiles = GROUPS[gi]
                g = len(tiles)
                j0 = tiles[0]
                C = c_tiles[gi]
                T8 = t8_tiles[gi]
                tslice = tau[:, j0:j0 + g]
                T3 = T8.rearrange("p (g k) -> p g k", g=g)
                hp = tc.high_priority()
                hp.__enter__()
                # tau0 = max(c1 - 1, (c1+c2)/2 - sqrt(1/2)) — sqrt-free lower bound
                tmp = pq.tile([P, g], f32, name=f"t0a_{gi}", tag=f"t0a_{gi}")
                nc.vector.tensor_tensor(
                    tmp.rearrange("p (g o) -> p g o", o=1),
                    T3[:, :, 0:1], T3[:, :, 1:2], Alu.add)
                nc.vector.tensor_scalar(tmp, tmp, 0.5, -0.70710678,
                                        Alu.mult, Alu.add)
                nc.vector.tensor_scalar(
                    tslice, T3[:, :, 0], -1.0, None, Alu.add)
                nc.vector.tensor_tensor(tslice, tslice, tmp, Alu.max)
                last = None
                for it in range(k_newton):
                    U = pq.tile([P, g * CAND], f16, name=f"U{gi}_{it}", tag="U")
                    for j, i in enumerate(tiles):
                        nc.vector.tensor_scalar(
                            U[:, j * CAND:(j + 1) * CAND],
                            C[:, j * CAND:(j + 1) * CAND], tau[:, i:i + 1], 0.0,
                            Alu.subtract, Alu.max)
                    SQ = pq.tile([P, g * CAND], f16, name=f"SQ{gi}_{it}", tag="SQ")
                    nc.vector.tensor_tensor(SQ, U, U, Alu.mult)
                    hF = pq.tile([P, 2 * g], f32, name=f"hF{gi}_{it}", tag="hF")
                    nc.vector.reduce_sum(
                        hF[:, 0:g].rearrange("p (g o) -> p g o", o=1),
                        U.rearrange("p (g c) -> p g c", g=g),
                        axis=mybir.AxisListType.X)
                    nc.vector.reduce_sum(
                        hF[:, g:2 * g].rearrange("p (g o) -> p g o", o=1),
                        SQ.rearrange("p (g c) -> p g c", g=g),
                        axis=mybir.AxisListType.X)
                    num = pq.tile([P, g], f32, name=f"num{gi}_{it}", tag="num")
                    nc.vector.tensor_scalar(num, hF[:, g:2 * g], -1.0, 0.5,
                                            Alu.add, Alu.mult)
                    rd = pq.tile([P, g], f32, name=f"rd{gi}_{it}", tag="rd")
                    nc.vector.reciprocal(rd, hF[:, 0:g])
                    nc.vector.tensor_tensor(num, num, rd, Alu.mult)
                    last = nc.vector.tensor_tensor(tslice, tslice, num, Alu.add)
                hp.__exit__(None, None, None)
                return last

            def final_tile(i, after=None):
                """U = relu(z - tau) on DVE (fp16 4x), square in-place ACT, store."""
                t_i = t_tiles[i]
                u = pu.tile([P, N_COLS], f16, name=f"u{i}", tag="u")
                relu = nc.vector.tensor_scalar(u, t_i, tau[:, i:i + 1], 0.0,
                                               Alu.subtract, Alu.max)
                if after is not None:
                    add_dep_helper(_raw(relu), _raw(after), sync=False,
                                   reason="pipeline order hint")
                nc.scalar.activation(u, u, Act.Square)
                nc.gpsimd.dma_start(out=pout[i * P:(i + 1) * P, :], in_=u)
                return relu

            group_alloc(0)
            group_alloc(1)
            for j, i in enumerate(GROUPS[0]):
                scan_tile(0, j, i)
            s0 = phase_solve(0, k_newton=3)
            r0 = final_tile(0, after=s0)
            r1 = final_tile(1, after=r0)
            scan_tile(1, 0, 4, after=r1)
            scan_tile(1, 1, 5)
            r2 = final_tile(2)
            r3 = final_tile(3)
            scan_tile(1, 2, 6, after=r3)
            scan_tile(1, 3, 7)
            s1 = phase_solve(1, k_newton=3)
            prev = s1
            for i in GROUPS[1]:
                prev = final_tile(i, after=prev)

    nc.compile()
    return nc


def _host_prep(scores, mask):
    s = np.asarray(scores, dtype=np.float32)
    zq = (np.float32(0.5) * s).astype(np.float16)
    z16 = np.where(np.asarray(mask), zq, np.float16(-4.0))
    return z16


def run(scores: np.ndarray, mask: np.ndarray, trace: bool = False, **kw):
    from concourse.bass_utils import run_bass_kernel_spmd

    assert scores.shape == (N_ROWS, N_COLS) and mask.shape == (N_ROWS, N_COLS)
    if "nc" not in _CACHE:
        _CACHE["nc"] = build_nc()
    nc = _CACHE["nc"]

    z16 = _host_prep(scores, mask)
    rpc = ROWS_PER_CORE
    in_maps = [
        {"z": np.ascontiguousarray(z16[i * rpc:(i + 1) * rpc])}
        for i in range(N_CORES)
    ]
    res = run_bass_kernel_spmd(nc, in_maps, list(range(N_CORES)), trace=trace, **kw)
    out = np.concatenate([res.results[i]["p"] for i in range(N_CORES)], axis=0)
    return np.ascontiguousarray(out.astype(np.float32)), res


def kernel(scores: np.ndarray, mask: np.ndarray) -> np.ndarray:
    return run(scores, mask)[0]


if __name__ == "__main__":
    rng = np.random.default_rng(0)
    scores = rng.standard_normal((N_ROWS, N_COLS), dtype=np.float32)
    mask = rng.integers(0, 2, (N_ROWS, N_COLS)).astype(bool)
    out = kernel(scores, mask)
    print("out", out.shape, out.dtype, "rowsum", out.sum(-1)[:4])


# revision 8
# speedup vs baseline: 1.0351x; 1.0116x over previous
"""Trainium2 Bass kernel v7 for entmax-1.5 over rows of a masked [8192, 4096] matrix.

Candidate-set Newton (32 candidates/row via double fold + chunked MAX8 top-8):
  - DVE: fold max(zL,zR) then fold again (fp16 2x), 4x MAX8@256 per tile ->
    C[128,32], merge MAX8 -> sorted T8 (c1,c2 for the start)
  - solve per group of 4 tiles: tau0 = max(c1-1, (c1+c2)/2 - sqrt(1/2))
    + K=3 Newton over candidates (pure DVE, high priority, no ACT deps)
  - finals: U = relu(z - tau) on DVE tensor_scalar (fp16 4x mode),
    p = U^2 on ACT (Square) except last tiles on DVE (breaks ACT tail ladder),
    full-tile DMA store
  - loads/stores on the Pool queue (SWDGE: ~0.34ns/descriptor vs ~25ns
    HWDGE) except the first two loads on SP for latency
Numpy-validated rel err ~1.01e-2 vs 2e-2 gate.

Sharding: 1024 rows x 8 cores; 8 tiles of [128, 4096] per core.
Self-contained: hardcodes scores[8192,4096] f32 + mask[8192,4096] bool.
"""

import sys

import numpy as np

sys.path.insert(0, "/opt/trn_rl_repo")

N_ROWS = 8192
N_COLS = 4096
N_CORES = 8
P = 128
ROWS_PER_CORE = N_ROWS // N_CORES          # 1024
NT = ROWS_PER_CORE // P                    # 8 tiles per core
NCH = 4
CAND = NCH * 8                             # 32 candidates per row
GROUPS = [(0, 1, 2, 3), (4, 5, 6, 7)]
DVE_SQ = (6, 7)                            # squares on DVE to break ACT tail ladder

_CACHE = {}


def build_nc():
    import concourse.bacc as bacc
    import concourse.mybir as mybir
    from concourse.tile import TileContext
    from concourse.tile_rust import add_dep_helper

    def _raw(x):
        for attr in ("ins", "instruction", "inst"):
            if hasattr(x, attr):
                return getattr(x, attr)
        return x

    f32 = mybir.dt.float32
    f16 = mybir.dt.float16
    Alu = mybir.AluOpType
    Act = mybir.ActivationFunctionType

    nc = bacc.Bacc("TRN2", target_bir_lowering=False, debug=False)

    z_h = nc.declare_dram_parameter("z", [ROWS_PER_CORE, N_COLS], f16, isOutput=False)
    p_h = nc.declare_dram_parameter("p", [ROWS_PER_CORE, N_COLS], f16, isOutput=True)

    z = z_h.ap()
    pout = p_h.ap()
    half = N_COLS // 2
    quart = N_COLS // 4
    csz = quart // NCH                     # 256

    with TileContext(nc) as tc:
        with (
            tc.tile_pool(name="pt", bufs=NT) as pt,
            tc.tile_pool(name="pw", bufs=2) as pw,
            tc.tile_pool(name="pw2", bufs=2) as pw2,
            tc.tile_pool(name="pu", bufs=5) as pu,
            tc.tile_pool(name="ps", bufs=1) as ps,
            tc.tile_pool(name="pq", bufs=4) as pq,
        ):
            tau = ps.tile([P, NT], f32, name="tau")

            t_tiles = [None] * NT
            c_tiles = {}
            t8_tiles = {}

            # ---- all input loads up front, in tile order ----
            # tiles 0,1 on the SP queue (low latency); rest on Pool/SWDGE
            for i in range(NT):
                t_i = pt.tile([P, N_COLS], f16, name=f"t{i}", tag="t")
                eng = nc.sync if i < 2 else nc.gpsimd
                eng.dma_start(out=t_i, in_=z[i * P:(i + 1) * P, :])
                t_tiles[i] = t_i

            def scan_tile(gi, j, i, after=None):
                """double fold + chunked MAX8 (all DVE) for tile i, slot j of group gi."""
                C = c_tiles[gi]
                T8 = t8_tiles[gi]
                t_i = t_tiles[i]
                w = pw.tile([P, half], f16, name=f"w{i}", tag="w")
                fold = nc.vector.tensor_tensor(w, t_i[:, :half], t_i[:, half:], Alu.max)
                if after is not None:
                    add_dep_helper(_raw(fold), _raw(after), sync=False,
                                   reason="pipeline order hint")
                w2 = pw2.tile([P, quart], f16, name=f"w2_{i}", tag="w2")
                nc.vector.tensor_tensor(w2, w[:, :quart], w[:, quart:], Alu.max)
                for c in range(NCH):
                    nc.vector.max(
                        C[:, j * CAND + c * 8: j * CAND + (c + 1) * 8],
                        w2[:, c * csz:(c + 1) * csz])
                nc.vector.max(T8[:, j * 8:(j + 1) * 8], C[:, j * CAND:(j + 1) * CAND])

            def group_alloc(gi):
                g = len(GROUPS[gi])
                c_tiles[gi] = ps.tile([P, g * CAND], f16, name=f"C{gi}")
                t8_tiles[gi] = ps.tile([P, g * 8], f32, name=f"T8_{gi}")

            def phase_solve(gi, k_newton):
                tiles = GROUPS[gi]
                g = len(tiles)
                j0 = tiles[0]
                C = c_tiles[gi]
                T8 = t8_tiles[gi]
                tslice = tau[:, j0:j0 + g]
                T3 = T8.rearrange("p (g k) -> p g k", g=g)
                hp = tc.high_priority()
                hp.__enter__()
                # tau0 = max(c1 - 1, (c1+c2)/2 - sqrt(1/2)) — sqrt-free lower bound
                tmp = pq.tile([P, g], f32, name=f"t0a_{gi}", tag=f"t0a_{gi}")
                nc.vector.tensor_tensor(
                    tmp.rearrange("p (g o) -> p g o", o=1),
                    T3[:, :, 0:1], T3[:, :, 1:2], Alu.add)
                nc.vector.tensor_scalar(tmp, tmp, 0.5, -0.70710678,
                                        Alu.mult, Alu.add)
                nc.vector.tensor_scalar(
                    tslice, T3[:, :, 0], -1.0, None, Alu.add)
                nc.vector.tensor_tensor(tslice, tslice, tmp, Alu.max)
                last = None
                for it in range(k_newton):
                    U = pq.tile([P, g * CAND], f16, name=f"U{gi}_{it}", tag="U")
                    for j, i in enumerate(tiles):
                        nc.vector.tensor_scalar(
                            U[:, j * CAND:(j + 1) * CAND],
                            C[:, j * CAND:(j + 1) * CAND], tau[:, i:i + 1], 0.0,
                            Alu.subtract, Alu.max)
                    SQ = pq.tile([P, g * CAND], f16, name=f"SQ{gi}_{it}", tag="SQ")
                    nc.vector.tensor_tensor(SQ, U, U, Alu.mult)
                    hF = pq.tile([P, 2 * g], f32, name=f"hF{gi}_{it}", tag="hF")
                    nc.vector.reduce_sum(
                        hF[:, 0:g].rearrange("p (g o) -> p g o", o=1),
                        U.rearrange("p (g c) -> p g c", g=g),
                        axis=mybir.AxisListType.X)
                    nc.vector.reduce_sum(
                        hF[:, g:2 * g].rearrange("p (g o) -> p g o", o=1),
                        SQ.rearrange("p (g c) -> p g c", g=g),
                        axis=mybir.AxisListType.X)
                    num = pq.tile([P, g], f32, name=f"num{gi}_{it}", tag="num")
                    nc.vector.tensor_scalar(num, hF[:, g:2 * g], -1.0, 0.5,
                                            Alu.add, Alu.mult)
                    rd = pq.tile([P, g], f32, name=f"rd{gi}_{it}", tag="rd")
                    nc.vector.reciprocal(rd, hF[:, 0:g])
                    nc.vector.tensor_tensor(num, num, rd, Alu.mult)
                    last = nc.vector.tensor_tensor(tslice, tslice, num, Alu.add)
                hp.__exit__(None, None, None)
                return last

            def final_tile(i, after=None):
                """U = relu(z - tau) on DVE (fp16 4x), square (ACT or DVE), store."""
                t_i = t_tiles[i]
                u = pu.tile([P, N_COLS], f16, name=f"u{i}", tag="u")
                relu = nc.vector.tensor_scalar(u, t_i, tau[:, i:i + 1], 0.0,
                                               Alu.subtract, Alu.max)
                if after is not None:
                    add_dep_helper(_raw(relu), _raw(after), sync=False,
                                   reason="pipeline order hint")
                if i in DVE_SQ:
                    nc.vector.tensor_tensor(u, u, u, Alu.mult)
                else:
                    nc.scalar.activation(u, u, Act.Square)
                nc.gpsimd.dma_start(out=pout[i * P:(i + 1) * P, :], in_=u)
                return relu

            group_alloc(0)
            group_alloc(1)
            for j, i in enumerate(GROUPS[0]):
                scan_tile(0, j, i)
            s0 = phase_solve(0, k_newton=3)
            r0 = final_tile(0, after=s0)
            r1 = final_tile(1, after=r0)
            scan_tile(1, 0, 4, after=r1)
            scan_tile(1, 1, 5)
            r2 = final_tile(2)
            r3 = final_tile(3)
            scan_tile(1, 2, 6, after=r3)
            scan_tile(1, 3, 7)
            s1 = phase_solve(1, k_newton=3)
            prev = s1
            for i in GROUPS[1]:
                prev = final_tile(i, after=prev)

    nc.compile()
    return nc


def _host_prep(scores, mask):
    s = np.asarray(scores, dtype=np.float32)
    zq = (np.float32(0.5) * s).astype(np.float16)
    z16 = np.where(np.asarray(mask), zq, np.float16(-4.0))
    return z16


def run(scores: np.ndarray, mask: np.ndarray, trace: bool = False, **kw):
    from concourse.bass_utils import run_bass_kernel_spmd

    assert scores.shape == (N_ROWS, N_COLS) and mask.shape == (N_ROWS, N_COLS)
    if "nc" not in _CACHE:
        _CACHE["nc"] = build_nc()
    nc = _CACHE["nc"]

    z16 = _host_prep(scores, mask)
    rpc = ROWS_PER_CORE
    in_maps = [
        {"z": np.ascontiguousarray(z16[i * rpc:(i + 1) * rpc])}
        for i in range(N_CORES)
    ]
    res = run_bass_kernel_spmd(nc, in_maps, list(range(N_CORES)), trace=trace, **kw)
    out = np.concatenate([res.results[i]["p"] for i in range(N_CORES)], axis=0)
    return np.ascontiguousarray(out.astype(np.float32)), res


def kernel(scores: np.ndarray, mask: np.ndarray) -> np.ndarray:
    return run(scores, mask)[0]


if __name__ == "__main__":
    rng = np.random.default_rng(0)
    scores = rng.standard_normal((N_ROWS, N_COLS), dtype=np.float32)
    mask = rng.integers(0, 2, (N_ROWS, N_COLS)).astype(bool)
    out = kernel(scores, mask)
    print("out", out.shape, out.dtype, "rowsum", out.sum(-1)[:4])


# revision 12
# speedup vs baseline: 1.0563x; 1.0205x over previous
"""Trainium2 Bass kernel v7 for entmax-1.5 over rows of a masked [8192, 4096] matrix.

Candidate-set Newton (32 candidates/row via double fold + chunked MAX8 top-8):
  - DVE: fold max(zL,zR) then fold again (fp16 2x), 4x MAX8@256 per tile ->
    C[128,32], merge MAX8 -> sorted T8 (c1,c2 for the start)
  - solve per group of 4 tiles: tau0 = max(c1-1, (c1+c2)/2 - sqrt(1/2))
    + K=3 Newton over candidates (pure DVE, high priority, no ACT deps)
  - finals: U = relu(z - tau) on DVE tensor_scalar (fp16 4x mode),
    p = U^2 on ACT (Square) except last tiles on DVE (breaks ACT tail ladder),
    full-tile DMA store
  - loads/stores on the Pool queue (SWDGE: ~0.34ns/descriptor vs ~25ns
    HWDGE) except the first two loads on SP for latency
Numpy-validated rel err ~1.01e-2 vs 2e-2 gate.

Sharding: 1024 rows x 8 cores; 8 tiles of [128, 4096] per core.
Self-contained: hardcodes scores[8192,4096] f32 + mask[8192,4096] bool.
"""

import sys

import numpy as np

sys.path.insert(0, "/opt/trn_rl_repo")

N_ROWS = 8192
N_COLS = 4096
N_CORES = 8
P = 128
ROWS_PER_CORE = N_ROWS // N_CORES          # 1024
NT = ROWS_PER_CORE // P                    # 8 tiles per core
NCH = 4
CAND = NCH * 8                             # 32 candidates per row
GROUPS = [(0, 1, 2, 3), (4, 5, 6, 7)]
DVE_SQ = (6, 7)                            # squares on DVE to break ACT tail ladder

_CACHE = {}


def build_nc():
    import concourse.bacc as bacc
    import concourse.mybir as mybir
    from concourse.tile import TileContext
    from concourse.tile_rust import add_dep_helper

    def _raw(x):
        for attr in ("ins", "instruction", "inst"):
            if hasattr(x, attr):
                return getattr(x, attr)
        return x

    f32 = mybir.dt.float32
    f16 = mybir.dt.float16
    Alu = mybir.AluOpType
    Act = mybir.ActivationFunctionType

    nc = bacc.Bacc("TRN2", target_bir_lowering=False, debug=False)

    z_h = nc.declare_dram_parameter("z", [ROWS_PER_CORE, N_COLS], f16, isOutput=False)
    p_h = nc.declare_dram_parameter("p", [ROWS_PER_CORE, N_COLS], f16, isOutput=True)

    z = z_h.ap()
    pout = p_h.ap()
    half = N_COLS // 2
    quart = N_COLS // 4
    csz = quart // NCH                     # 256

    with TileContext(nc) as tc:
        with (
            tc.tile_pool(name="pt", bufs=NT) as pt,
            tc.tile_pool(name="pw", bufs=2) as pw,
            tc.tile_pool(name="pw2", bufs=2) as pw2,
            tc.tile_pool(name="pu", bufs=5) as pu,
            tc.tile_pool(name="ps", bufs=1) as ps,
            tc.tile_pool(name="pq", bufs=4) as pq,
        ):
            tau = ps.tile([P, NT], f32, name="tau")

            t_tiles = [None] * NT
            c_tiles = {}
            t8_tiles = {}

            # ---- all input loads up front, in tile order ----
            # Row-half loads (64 descriptors each, halves HWDGE gen latency).
            # Tiles 0,1 split across SP+ACT queues (lowest latency); rest on
            # Pool/SWDGE (fast desc gen, keeps HBM streaming in tile order).
            H = P // 2
            for i in range(NT):
                t_i = pt.tile([P, N_COLS], f16, name=f"t{i}", tag="t")
                r0 = i * P
                if i < 2:
                    nc.sync.dma_start(out=t_i[0:H, :], in_=z[r0:r0 + H, :])
                    nc.scalar.dma_start(out=t_i[H:P, :], in_=z[r0 + H:r0 + P, :])
                else:
                    nc.gpsimd.dma_start(out=t_i[0:H, :], in_=z[r0:r0 + H, :])
                    nc.gpsimd.dma_start(out=t_i[H:P, :], in_=z[r0 + H:r0 + P, :])
                t_tiles[i] = t_i

            def scan_tile(gi, j, i, after=None):
                """double fold + chunked MAX8 (all DVE) for tile i, slot j of group gi."""
                C = c_tiles[gi]
                T8 = t8_tiles[gi]
                t_i = t_tiles[i]
                w = pw.tile([P, half], f16, name=f"w{i}", tag="w")
                fold = nc.vector.tensor_tensor(w, t_i[:, :half], t_i[:, half:], Alu.max)
                if after is not None:
                    add_dep_helper(_raw(fold), _raw(after), sync=False,
                                   reason="pipeline order hint")
                w2 = pw2.tile([P, quart], f16, name=f"w2_{i}", tag="w2")
                nc.vector.tensor_tensor(w2, w[:, :quart], w[:, quart:], Alu.max)
                for c in range(NCH):
                    nc.vector.max(
                        C[:, j * CAND + c * 8: j * CAND + (c + 1) * 8],
                        w2[:, c * csz:(c + 1) * csz])
                nc.vector.max(T8[:, j * 8:(j + 1) * 8], C[:, j * CAND:(j + 1) * CAND])

            def group_alloc(gi):
                g = len(GROUPS[gi])
                c_tiles[gi] = ps.tile([P, g * CAND], f16, name=f"C{gi}")
                t8_tiles[gi] = ps.tile([P, g * 8], f32, name=f"T8_{gi}")

            def phase_solve(gi, k_newton):
                tiles = GROUPS[gi]
                g = len(tiles)
                j0 = tiles[0]
                C = c_tiles[gi]
                T8 = t8_tiles[gi]
                tslice = tau[:, j0:j0 + g]
                T3 = T8.rearrange("p (g k) -> p g k", g=g)
                C3 = C.rearrange("p (g c) -> p g c", g=g)
                tauB = tslice.rearrange("p (g o) -> p g o", o=1).broadcast_to(
                    [P, g, CAND])
                hp = tc.high_priority()
                hp.__enter__()
                # tau0 = max(c1 - 1, (c1+c2)/2 - sqrt(1/2)) — sqrt-free lower bound
                tmp = pq.tile([P, g], f32, name=f"t0a_{gi}", tag=f"t0a_{gi}")
                nc.vector.tensor_tensor(
                    tmp.rearrange("p (g o) -> p g o", o=1),
                    T3[:, :, 0:1], T3[:, :, 1:2], Alu.add)
                nc.vector.tensor_scalar(tmp, tmp, 0.5, -0.70710678,
                                        Alu.mult, Alu.add)
                nc.vector.scalar_tensor_tensor(
                    tslice, T3[:, :, 0], -1.0, tmp, Alu.add, Alu.max)
                last = None
                for it in range(k_newton):
                    D = pq.tile([P, g * CAND], f16, name=f"D{gi}_{it}", tag="D")
                    D3 = D.rearrange("p (g c) -> p g c", g=g)
                    nc.vector.tensor_tensor(D3, C3, tauB, Alu.subtract)
                    U = pq.tile([P, g * CAND], f16, name=f"U{gi}_{it}", tag="U")
                    nc.vector.tensor_scalar(U, D, 0.0, None, Alu.max)
                    SQ = pq.tile([P, g * CAND], f16, name=f"SQ{gi}_{it}", tag="SQ")
                    nc.vector.tensor_tensor(SQ, U, U, Alu.mult)
                    hF = pq.tile([P, 2 * g], f32, name=f"hF{gi}_{it}", tag="hF")
                    nc.vector.reduce_sum(
                        hF[:, 0:g].rearrange("p (g o) -> p g o", o=1),
                        U.rearrange("p (g c) -> p g c", g=g),
                        axis=mybir.AxisListType.X)
                    nc.vector.reduce_sum(
                        hF[:, g:2 * g].rearrange("p (g o) -> p g o", o=1),
                        SQ.rearrange("p (g c) -> p g c", g=g),
                        axis=mybir.AxisListType.X)
                    num = pq.tile([P, g], f32, name=f"num{gi}_{it}", tag="num")
                    nc.vector.tensor_scalar(num, hF[:, g:2 * g], -1.0, 0.5,
                                            Alu.add, Alu.mult)
                    rd = pq.tile([P, g], f32, name=f"rd{gi}_{it}", tag="rd")
                    nc.vector.reciprocal(rd, hF[:, 0:g])
                    nc.vector.tensor_tensor(num, num, rd, Alu.mult)
                    last = nc.vector.tensor_tensor(tslice, tslice, num, Alu.add)
                hp.__exit__(None, None, None)
                return last

            nega = ps.tile([P, 2], f32, name="nega")

            def final_tile(i, after=None, act_relu=False):
                """U = relu(z - tau) (DVE fp16 4x or ACT), square (ACT or DVE), store."""
                t_i = t_tiles[i]
                u = pu.tile([P, N_COLS], f16, name=f"u{i}", tag="u")
                if act_relu:
                    relu = nc.scalar.activation(u, t_i, Act.Relu,
                                                bias=nega[:, i - 2:i - 1], scale=1.0)
                else:
                    relu = nc.vector.tensor_scalar(u, t_i, tau[:, i:i + 1], 0.0,
                                                   Alu.subtract, Alu.max)
                if after is not None:
                    add_dep_helper(_raw(relu), _raw(after), sync=False,
                                   reason="pipeline order hint")
                if i in DVE_SQ:
                    nc.vector.tensor_tensor(u, u, u, Alu.mult)
                else:
                    nc.scalar.activation(u, u, Act.Square)
                nc.gpsimd.dma_start(out=pout[i * P:(i + 1) * P, :], in_=u)
                return relu

            group_alloc(0)
            group_alloc(1)
            for j, i in enumerate(GROUPS[0]):
                scan_tile(0, j, i)
            s0 = phase_solve(0, k_newton=3)
            # nega for ACT-relu of tiles 2,3 (bias = -tau)
            nga = nc.vector.tensor_scalar(nega, tau[:, 2:4], -1.0, None, Alu.mult)
            r0 = final_tile(0, after=nga)
            r1 = final_tile(1, after=r0)
            scan_tile(1, 0, 4, after=r1)
            scan_tile(1, 1, 5)
            final_tile(2, act_relu=True)
            final_tile(3, act_relu=True)
            scan_tile(1, 2, 6)
            scan_tile(1, 3, 7)
            s1 = phase_solve(1, k_newton=3)
            prev = s1
            for i in GROUPS[1]:
                prev = final_tile(i, after=prev)

    nc.compile()
    return nc


def _host_prep(scores, mask):
    s = np.asarray(scores, dtype=np.float32)
    zq = (np.float32(0.5) * s).astype(np.float16)
    z16 = np.where(np.asarray(mask), zq, np.float16(-4.0))
    return z16


def run(scores: np.ndarray, mask: np.ndarray, trace: bool = False, **kw):
    from concourse.bass_utils import run_bass_kernel_spmd

    assert scores.shape == (N_ROWS, N_COLS) and mask.shape == (N_ROWS, N_COLS)
    if "nc" not in _CACHE:
        _CACHE["nc"] = build_nc()
    nc = _CACHE["nc"]

    z16 = _host_prep(scores, mask)
    rpc = ROWS_PER_CORE
    in_maps = [
        {"z": np.ascontiguousarray(z16[i * rpc:(i + 1) * rpc])}
        for i in range(N_CORES)
    ]
    res = run_bass_kernel_spmd(nc, in_maps, list(range(N_CORES)), trace=trace, **kw)
    out = np.concatenate([res.results[i]["p"] for i in range(N_CORES)], axis=0)
    return np.ascontiguousarray(out.astype(np.float32)), res


def kernel(scores: np.ndarray, mask: np.ndarray) -> np.ndarray:
    return run(scores, mask)[0]


if __name__ == "__main__":
    rng = np.random.default_rng(0)
    scores = rng.standard_normal((N_ROWS, N_COLS), dtype=np.float32)
    mask = rng.integers(0, 2, (N_ROWS, N_COLS)).astype(bool)
    out = kernel(scores, mask)
    print("out", out.shape, out.dtype, "rowsum", out.sum(-1)[:4])
